# revision 14
# baseline (speedup 1.0000x reference)
"""Banded-matrix matmul kernel for Trainium2, SPMD over 8 NeuronCores.

Problem: out[b,s,o] = sum_i x[b,s,i] * W[o,i] + bias[o] with W a 4096x4096
band matrix (bandwidth 512 -> W[o,i] != 0 iff |o-i| <= 512), given in COO
form (W_values, rows, cols) with deterministic band ordering.

Strategy:
  - Host: densify W; shard tokens 8-way (data parallel; band + bias
    replicated). All device-side tensors are host-packed partition-major so
    every DMA is a 2D pattern with 8-18KB contiguous per-partition rows
    (SDMA packet overhead amortized; the band's zero padding never moves).
  - Device (per core): out.T[o,s] = W @ x.T per 128-row output tile,
    accumulating over the band's k-tiles in PSUM via float32r TensorEngine
    matmuls (full fp32 data, 1 cycle/row at N=512), bias added during the
    PSUM->SBUF drain on DVE/ACT. x streams through a sliding window of
    4-k-tile groups; W streams in 4-output-tile groups; outputs leave in
    2-output-tile stores.
  - Host: unpack per-core [128, 32*1024] outputs back to [B, S, 4096].
"""

import sys

if "/opt/trn_rl_repo" not in sys.path:
    sys.path.insert(0, "/opt/trn_rl_repo")

import numpy as np

import concourse.bass as bass
import concourse.mybir as mybir
from concourse import tile
from concourse import bass_utils
from concourse.vector_clock import ScopedClock
from concourse.bass_utils import run_bass_kernel_spmd

# ---------------------------------------------------------------- constants
N_CORES = 8
NIN = 4096
NOUT = 4096
BW = 512
B, S = 4, 2048
TOK = B * S            # 8192 tokens
TPC = TOK // N_CORES   # 1024 tokens per core
P = 128                # partitions
NT = NOUT // P         # 32 output tiles of 128 rows
HALF = 512             # moving-operand free size per matmul (4-byte max)

XG = 4                 # k-tiles per x-group       (16KB/partition rows)
WG = 4                 # o-tiles per weight group  (<=18.4KB/partition rows)
OG = 2                 # o-tiles per output store  (8KB/partition rows)
NXG = NT // XG
NWG = NT // WG

# per output tile t: band spans k-tiles [KS[t], KE[t])
KS = [max(0, t - BW // P) for t in range(NT)]
KE = [min(NT, t + BW // P + 1) for t in range(NT)]
NK = [KE[t] - KS[t] for t in range(NT)]
# weight-group layout: group g holds o-tiles [g*WG, (g+1)*WG), each slab
# [P, nk*P] partition-major, concatenated along the free axis
WGNK = [sum(NK[g * WG + i] for i in range(WG)) for g in range(NWG)]
WGOFF = [0] * NWG
for g in range(1, NWG):
    WGOFF[g] = WGOFF[g - 1] + WGNK[g - 1]
WGNK_MAX = max(WGNK)
NK_TOTAL = sum(NK)

COMPUTE_DT = mybir.dt.float32r  # fp32 data, full-rate PE mode
OUT_DT = mybir.dt.float32

# ------------------------------------------------- walrus 1-wait workaround
_MAXW = 1


def _split_drain_and_barrier(self, tick_clock, wait_clock):
    nc = self.nc
    probe = nc.sync.nop(nofuse=True, hint="pre_drain_waits")
    wait_clock.add_sem_waits(probe.ins, ScopedClock({None: tick_clock.global_clock}))
    si = probe.ins.sync_info
    waits = list(si.on_wait) if si is not None and si.on_wait else []
    if len(waits) > _MAXW:
        probe.ins.sync_info = mybir.SyncInfo(
            on_wait=waits[:_MAXW],
            on_update=list(si.on_update) if si.on_update else [],
        )
        for i in range(_MAXW, len(waits), _MAXW):
            extra = nc.sync.nop(nofuse=True, hint=f"pre_drain_waits_{i}")
            extra.ins.sync_info = mybir.SyncInfo(
                on_wait=waits[i : i + _MAXW], on_update=[]
            )
    drain_inst = nc.sync.drain()
    wait_clock.add_sem_waits(
        drain_inst.ins, ScopedClock({None: tick_clock.global_clock})
    )
    dsi = drain_inst.ins.sync_info
    dwaits = list(dsi.on_wait) if dsi is not None and dsi.on_wait else []
    if len(dwaits) > _MAXW:
        # the NOPs above ran earlier on the same sequencer and carried them all
        drain_inst.ins.sync_info = mybir.SyncInfo(
            on_wait=[], on_update=list(dsi.on_update) if dsi.on_update else []
        )
    nc.all_engine_barrier()
    popped = nc._tile_sem_poison_stack.pop()
    assert popped is self._sem_poison
    nc.clear_and_free_semaphores(list(self.sems.allocated().values()))
    nc.all_engine_barrier()


tile.TileContext._drain_and_barrier = _split_drain_and_barrier


def fix_multi_waits(nc: bass.Bass) -> None:
    """This walrus build allows only ONE sync wait per instruction. Carry
    extra waits on single-wait NOPs inserted just before, on the same
    engine/sequencer."""
    for bb in nc.m.functions[0].blocks:
        changed = False
        new_insts = []
        for inst in bb.instructions:
            si = inst.sync_info
            waits = list(si.on_wait) if si is not None and si.on_wait else []
            if len(waits) > 1:
                for w in waits[:-1]:
                    nop = mybir.InstNoOp(
                        name=nc.get_next_instruction_name(),
                        engine=inst.engine,
                        bass_nofuse=True,
                        sync_info=mybir.SyncInfo(on_wait=[w], on_update=[]),
                    )
                    new_insts.append(nop)
                inst.sync_info = mybir.SyncInfo(
                    on_wait=[waits[-1]],
                    on_update=list(si.on_update) if si.on_update else [],
                )
                changed = True
            new_insts.append(inst)
        if changed:
            bb.instructions = new_insts


# upload_artifacts reaches an internal blob store not present here; the trace
# path only needs the local files.
bass_utils.upload_artifacts = lambda tmpdir: "local://" + tmpdir


# ---------------------------------------------------------------- device IR
def build_program() -> bass.Bass:
    nc = bass.Bass()
    # all host-packed partition-major (see kernel())
    xpk = nc.declare_dram_parameter("xpk", [P, NT * TPC], COMPUTE_DT, isOutput=False)
    wpk = nc.declare_dram_parameter("wpk", [P, NK_TOTAL * P], COMPUTE_DT, isOutput=False)
    bias = nc.declare_dram_parameter("bias_pk", [P, NT], OUT_DT, isOutput=False)
    outp = nc.declare_dram_parameter("outpk", [P, NT * TPC], OUT_DT, isOutput=True)

    with tile.TileContext(nc) as tc:
        with (
            tc.tile_pool(name="xp", bufs=5) as xp,
            tc.tile_pool(name="wp", bufs=4) as wp,
            tc.tile_pool(name="op", bufs=3) as op,
            tc.tile_pool(name="bp", bufs=1) as bp,
            tc.tile_pool(name="pp", bufs=4, space="PSUM") as pp,
        ):
            bias_sb = bp.tile([P, NT], OUT_DT)
            nc.sync.dma_start(out=bias_sb[:, :], in_=bias[:, :])

            x_tiles: list = [None] * NXG
            w_tiles: list = [None] * NWG

            def load_xg(g):
                # two half-loads so consumers of early k-tiles start sooner
                # (Tile tracks subtile deps per DMA write range)
                xt = xp.tile([P, XG * TPC], COMPUTE_DT, tag="xg", name=f"xg{g}")
                h = XG * TPC // 2
                base = g * XG * TPC
                nc.sync.dma_start(out=xt[:, 0:h], in_=xpk[:, base : base + h])
                nc.sync.dma_start(
                    out=xt[:, h : 2 * h], in_=xpk[:, base + h : base + 2 * h]
                )
                x_tiles[g] = xt

            def load_wg(g):
                wt = wp.tile(
                    [P, WGNK[g] * P], COMPUTE_DT, tag="wg", name=f"wg{g}",
                    padded_shape=[P, WGNK_MAX * P],
                )
                half_tiles = WGNK[g] // 2
                h = half_tiles * P
                base = WGOFF[g] * P
                full = WGNK[g] * P
                nc.sync.dma_start(out=wt[:, 0:h], in_=wpk[:, base : base + h])
                nc.sync.dma_start(
                    out=wt[:, h:full], in_=wpk[:, base + h : base + full]
                )
                w_tiles[g] = wt

            ot = None
            for t in range(NT):
                gw = t // WG
                if w_tiles[gw] is None:
                    load_wg(gw)
                for g in range(KS[t] // XG, (KE[t] - 1) // XG + 1):
                    if x_tiles[g] is None:
                        load_xg(g)

                # slab offset of o-tile t inside its weight group
                off = sum(NK[gw * WG + i] for i in range(t - gw * WG))
                wt = w_tiles[gw]

                ps0 = pp.tile([P, HALF], mybir.dt.float32, name=f"ps0_{t}", tag="ps0")
                ps1 = pp.tile([P, HALF], mybir.dt.float32, name=f"ps1_{t}", tag="ps1")
                for j in range(NK[t]):
                    k = KS[t] + j
                    lhsT = wt[:, (off + j) * P : (off + j + 1) * P]
                    xg = x_tiles[k // XG]
                    xbase = (k % XG) * TPC
                    nc.tensor.matmul(
                        ps0[:, :], lhsT, xg[:, xbase : xbase + HALF],
                        start=(j == 0), stop=(j == NK[t] - 1),
                    )
                    nc.tensor.matmul(
                        ps1[:, :], lhsT, xg[:, xbase + HALF : xbase + TPC],
                        start=(j == 0), stop=(j == NK[t] - 1),
                    )

                if t % OG == 0:
                    ot = op.tile([P, OG * TPC], OUT_DT, name=f"ot{t}", tag="ot")
                obase = (t % OG) * TPC
                bias_col = bias_sb[:, t : t + 1]
                nc.vector.tensor_scalar_add(
                    ot[:, obase : obase + HALF], ps0[:, :], bias_col
                )
                nc.scalar.activation(
                    ot[:, obase + HALF : obase + TPC], ps1[:, :],
                    mybir.ActivationFunctionType.Identity, bias=bias_col,
                )
                if t % OG == OG - 1:
                    nc.sync.dma_start(
                        out=outp[:, (t - OG + 1) * TPC : (t + 1) * TPC],
                        in_=ot[:, :],
                    )

    fix_multi_waits(nc)
    return nc


_PROGRAM_CACHE: bass.Bass | None = None


def _program() -> bass.Bass:
    global _PROGRAM_CACHE
    if _PROGRAM_CACHE is None:
        _PROGRAM_CACHE = build_program()
    return _PROGRAM_CACHE


# --------------------------------------------------------------- host side
def _pack_weights(W_values, rows, cols) -> np.ndarray:
    W = np.zeros((NOUT, NIN), dtype=np.float32)
    W[rows, cols] = W_values
    slabs = []
    for t in range(NT):
        # slab[p, j*P + o] = W[t*P + o, (KS[t]+j)*P + p]
        blk = W[t * P : (t + 1) * P, KS[t] * P : KE[t] * P]  # [o, nk*P]
        slab = blk.reshape(P, NK[t], P).transpose(2, 1, 0).reshape(P, NK[t] * P)
        slabs.append(slab)
    return np.ascontiguousarray(np.concatenate(slabs, axis=1))  # [P, NK_TOTAL*P]


def kernel(x, W_values, bias, rows, cols, _trace=False):
    x = np.asarray(x, dtype=np.float32)
    W_values = np.asarray(W_values, dtype=np.float32)
    bias = np.asarray(bias, dtype=np.float32)
    rows = np.asarray(rows)
    cols = np.asarray(cols)

    x2d = x.reshape(TOK, NIN)
    wpk = _pack_weights(W_values, rows, cols)
    bias_pk = np.ascontiguousarray(bias.reshape(NT, P).T)

    in_maps = []
    for c in range(N_CORES):
        xs = x2d[c * TPC : (c + 1) * TPC, :]  # [TPC, NIN]
        # xpk[p, j*TPC + s] = xs[s, j*P + p]
        xpk = np.ascontiguousarray(
            xs.reshape(TPC, NT, P).transpose(2, 1, 0).reshape(P, NT * TPC)
        )
        in_maps.append({"xpk": xpk, "wpk": wpk, "bias_pk": bias_pk})

    nc = _program()
    res = run_bass_kernel_spmd(
        nc, in_maps, core_ids=list(range(N_CORES)), trace=_trace,
        trace_cores=list(range(N_CORES)) if _trace else None,
    )

    out = np.empty((TOK, NOUT), dtype=np.float32)
    for c in range(N_CORES):
        outpk = res.results[c]["outpk"]  # [P, NT*TPC]
        # out[s, t*P + p] = outpk[p, t*TPC + s]
        out[c * TPC : (c + 1) * TPC, :] = (
            outpk.reshape(P, NT, TPC).transpose(2, 1, 0).reshape(TPC, NOUT)
        )
    out = out.reshape(B, S, NOUT)

    if _trace:
        kernel.last_exec_time_ns = res.exec_time_ns
        kernel.last_results = res
    return out


# revision 18
# speedup vs baseline: 1.3092x; 1.3092x over previous
"""Banded-matrix matmul kernel for Trainium2, SPMD over 8 NeuronCores.

Problem: out[b,s,o] = sum_i x[b,s,i] * W[o,i] + bias[o] with W a 4096x4096
band matrix (bandwidth 512 -> W[o,i] != 0 iff |o-i| <= 512), given in COO
form (W_values, rows, cols) with deterministic band ordering.

Strategy:
  - Host: densify W; shard tokens 8-way (data parallel; band + bias
    replicated). All device-side tensors are host-packed partition-major so
    every DMA is a 2D pattern with 8-18KB contiguous per-partition rows
    (SDMA packet overhead amortized; the band's zero padding never moves).
  - Device (per core): out.T[o,s] = W @ x.T per 128-row output tile,
    accumulating over the band's k-tiles in PSUM via float32r TensorEngine
    matmuls (full fp32 data, 1 cycle/row at N=512), bias added during the
    PSUM->SBUF drain on DVE/ACT. x streams through a sliding window of
    4-k-tile groups; W streams in 4-output-tile groups; outputs leave in
    2-output-tile stores.
  - Host: unpack per-core [128, 32*1024] outputs back to [B, S, 4096].
"""

import sys

if "/opt/trn_rl_repo" not in sys.path:
    sys.path.insert(0, "/opt/trn_rl_repo")

import numpy as np

import concourse.bass as bass
import concourse.mybir as mybir
from concourse import tile
from concourse import bass_utils
from concourse.vector_clock import ScopedClock
from concourse.bass_utils import run_bass_kernel_spmd

# ---------------------------------------------------------------- constants
N_CORES = 8
NIN = 4096
NOUT = 4096
BW = 512
B, S = 4, 2048
TOK = B * S            # 8192 tokens
TPC = TOK // N_CORES   # 1024 tokens per core
P = 128                # partitions
NT = NOUT // P         # 32 output tiles of 128 rows
HALF = 512             # moving-operand free size per matmul (4-byte max)

XG = 4                 # k-tiles per x-group       (16KB/partition rows)
WG = 4                 # o-tiles per weight group  (<=18.4KB/partition rows)
OG = 2                 # o-tiles per output store  (8KB/partition rows)
NXG = NT // XG
NWG = NT // WG

# per output tile t: band spans k-tiles [KS[t], KE[t])
KS = [max(0, t - BW // P) for t in range(NT)]
KE = [min(NT, t + BW // P + 1) for t in range(NT)]
NK = [KE[t] - KS[t] for t in range(NT)]
# weight-group layout: group g holds o-tiles [g*WG, (g+1)*WG), each slab
# [P, nk*P] partition-major, concatenated along the free axis
WGNK = [sum(NK[g * WG + i] for i in range(WG)) for g in range(NWG)]
WGOFF = [0] * NWG
for g in range(1, NWG):
    WGOFF[g] = WGOFF[g - 1] + WGNK[g - 1]
WGNK_MAX = max(WGNK)
NK_TOTAL = sum(NK)

COMPUTE_DT = mybir.dt.float16   # halves x/W HBM traffic; ~5e-4 rounding,
                                # fp32 PSUM accumulation; 1 PE cycle/row
COMPUTE_NP = np.float16
OUT_DT = mybir.dt.float32

# ------------------------------------------------- walrus 1-wait workaround
_MAXW = 1


def _split_drain_and_barrier(self, tick_clock, wait_clock):
    nc = self.nc
    probe = nc.sync.nop(nofuse=True, hint="pre_drain_waits")
    wait_clock.add_sem_waits(probe.ins, ScopedClock({None: tick_clock.global_clock}))
    si = probe.ins.sync_info
    waits = list(si.on_wait) if si is not None and si.on_wait else []
    if len(waits) > _MAXW:
        probe.ins.sync_info = mybir.SyncInfo(
            on_wait=waits[:_MAXW],
            on_update=list(si.on_update) if si.on_update else [],
        )
        for i in range(_MAXW, len(waits), _MAXW):
            extra = nc.sync.nop(nofuse=True, hint=f"pre_drain_waits_{i}")
            extra.ins.sync_info = mybir.SyncInfo(
                on_wait=waits[i : i + _MAXW], on_update=[]
            )
    drain_inst = nc.sync.drain()
    wait_clock.add_sem_waits(
        drain_inst.ins, ScopedClock({None: tick_clock.global_clock})
    )
    dsi = drain_inst.ins.sync_info
    dwaits = list(dsi.on_wait) if dsi is not None and dsi.on_wait else []
    if len(dwaits) > _MAXW:
        # the NOPs above ran earlier on the same sequencer and carried them all
        drain_inst.ins.sync_info = mybir.SyncInfo(
            on_wait=[], on_update=list(dsi.on_update) if dsi.on_update else []
        )
    nc.all_engine_barrier()
    popped = nc._tile_sem_poison_stack.pop()
    assert popped is self._sem_poison
    nc.clear_and_free_semaphores(list(self.sems.allocated().values()))
    nc.all_engine_barrier()


tile.TileContext._drain_and_barrier = _split_drain_and_barrier


def fix_multi_waits(nc: bass.Bass) -> None:
    """This walrus build allows only ONE sync wait per instruction. Carry
    extra waits on single-wait NOPs inserted just before, on the same
    engine/sequencer."""
    for bb in nc.m.functions[0].blocks:
        changed = False
        new_insts = []
        for inst in bb.instructions:
            si = inst.sync_info
            waits = list(si.on_wait) if si is not None and si.on_wait else []
            if len(waits) > 1:
                for w in waits[:-1]:
                    nop = mybir.InstNoOp(
                        name=nc.get_next_instruction_name(),
                        engine=inst.engine,
                        bass_nofuse=True,
                        sync_info=mybir.SyncInfo(on_wait=[w], on_update=[]),
                    )
                    new_insts.append(nop)
                inst.sync_info = mybir.SyncInfo(
                    on_wait=[waits[-1]],
                    on_update=list(si.on_update) if si.on_update else [],
                )
                changed = True
            new_insts.append(inst)
        if changed:
            bb.instructions = new_insts


# upload_artifacts reaches an internal blob store not present here; the trace
# path only needs the local files.
bass_utils.upload_artifacts = lambda tmpdir: "local://" + tmpdir


# ---------------------------------------------------------------- device IR
def build_program() -> bass.Bass:
    nc = bass.Bass()
    # all host-packed partition-major (see kernel())
    xpk = nc.declare_dram_parameter("xpk", [P, NT * TPC], COMPUTE_DT, isOutput=False)
    wpk = nc.declare_dram_parameter("wpk", [P, NK_TOTAL * P], COMPUTE_DT, isOutput=False)
    bias = nc.declare_dram_parameter("bias_pk", [P, NT], OUT_DT, isOutput=False)
    outp = nc.declare_dram_parameter("outpk", [P, NT * TPC], OUT_DT, isOutput=True)

    with tile.TileContext(nc) as tc:
        with (
            # fp16 x and W fit SBUF-resident; one buffer per group, no reuse
            tc.tile_pool(name="xp", bufs=1) as xp,
            tc.tile_pool(name="wp", bufs=1) as wp,
            tc.tile_pool(name="op", bufs=3) as op,
            tc.tile_pool(name="bp", bufs=1) as bp,
            tc.tile_pool(name="pp", bufs=4, space="PSUM") as pp,
        ):
            bias_sb = bp.tile([P, NT], OUT_DT)
            nc.sync.dma_start(out=bias_sb[:, :], in_=bias[:, :])

            x_tiles: list = [None] * NXG
            w_tiles: list = [None] * NWG

            def load_xg(g):
                xt = xp.tile([P, XG * TPC], COMPUTE_DT, tag=f"xg{g}", name=f"xg{g}")
                nc.sync.dma_start(
                    out=xt[:, :], in_=xpk[:, g * XG * TPC : (g + 1) * XG * TPC]
                )
                x_tiles[g] = xt

            def load_wg(g):
                wt = wp.tile(
                    [P, WGNK[g] * P], COMPUTE_DT, tag=f"wg{g}", name=f"wg{g}",
                )
                nc.sync.dma_start(
                    out=wt[:, :],
                    in_=wpk[:, WGOFF[g] * P : (WGOFF[g] + WGNK[g]) * P],
                )
                w_tiles[g] = wt

            ot = None
            for t in range(NT):
                gw = t // WG
                if w_tiles[gw] is None:
                    load_wg(gw)
                for g in range(KS[t] // XG, (KE[t] - 1) // XG + 1):
                    if x_tiles[g] is None:
                        load_xg(g)

                # slab offset of o-tile t inside its weight group
                off = sum(NK[gw * WG + i] for i in range(t - gw * WG))
                wt = w_tiles[gw]

                ps0 = pp.tile([P, HALF], mybir.dt.float32, name=f"ps0_{t}", tag="ps0")
                ps1 = pp.tile([P, HALF], mybir.dt.float32, name=f"ps1_{t}", tag="ps1")
                for j in range(NK[t]):
                    k = KS[t] + j
                    lhsT = wt[:, (off + j) * P : (off + j + 1) * P]
                    xg = x_tiles[k // XG]
                    xbase = (k % XG) * TPC
                    nc.tensor.matmul(
                        ps0[:, :], lhsT, xg[:, xbase : xbase + HALF],
                        start=(j == 0), stop=(j == NK[t] - 1),
                    )
                    nc.tensor.matmul(
                        ps1[:, :], lhsT, xg[:, xbase + HALF : xbase + TPC],
                        start=(j == 0), stop=(j == NK[t] - 1),
                    )

                if t % OG == 0:
                    ot = op.tile([P, OG * TPC], OUT_DT, name=f"ot{t}", tag="ot")
                obase = (t % OG) * TPC
                bias_col = bias_sb[:, t : t + 1]
                nc.vector.tensor_scalar_add(
                    ot[:, obase : obase + HALF], ps0[:, :], bias_col
                )
                nc.scalar.activation(
                    ot[:, obase + HALF : obase + TPC], ps1[:, :],
                    mybir.ActivationFunctionType.Identity, bias=bias_col,
                )
                if t % OG == OG - 1:
                    nc.sync.dma_start(
                        out=outp[:, (t - OG + 1) * TPC : (t + 1) * TPC],
                        in_=ot[:, :],
                    )

    fix_multi_waits(nc)
    return nc


_PROGRAM_CACHE: bass.Bass | None = None


def _program() -> bass.Bass:
    global _PROGRAM_CACHE
    if _PROGRAM_CACHE is None:
        _PROGRAM_CACHE = build_program()
    return _PROGRAM_CACHE


# --------------------------------------------------------------- host side
def _pack_weights(W_values, rows, cols) -> np.ndarray:
    W = np.zeros((NOUT, NIN), dtype=np.float32)
    W[rows, cols] = W_values
    slabs = []
    for t in range(NT):
        # slab[p, j*P + o] = W[t*P + o, (KS[t]+j)*P + p]
        blk = W[t * P : (t + 1) * P, KS[t] * P : KE[t] * P]  # [o, nk*P]
        slab = blk.reshape(P, NK[t], P).transpose(2, 1, 0).reshape(P, NK[t] * P)
        slabs.append(slab)
    return np.ascontiguousarray(
        np.concatenate(slabs, axis=1), dtype=COMPUTE_NP
    )  # [P, NK_TOTAL*P]


def kernel(x, W_values, bias, rows, cols, _trace=False):
    x = np.asarray(x, dtype=np.float32)
    W_values = np.asarray(W_values, dtype=np.float32)
    bias = np.asarray(bias, dtype=np.float32)
    rows = np.asarray(rows)
    cols = np.asarray(cols)

    x2d = x.reshape(TOK, NIN)
    wpk = _pack_weights(W_values, rows, cols)
    bias_pk = np.ascontiguousarray(bias.reshape(NT, P).T)

    in_maps = []
    for c in range(N_CORES):
        xs = x2d[c * TPC : (c + 1) * TPC, :]  # [TPC, NIN]
        # xpk[p, j*TPC + s] = xs[s, j*P + p]
        xpk = np.ascontiguousarray(
            xs.reshape(TPC, NT, P).transpose(2, 1, 0).reshape(P, NT * TPC),
            dtype=COMPUTE_NP,
        )
        in_maps.append({"xpk": xpk, "wpk": wpk, "bias_pk": bias_pk})

    nc = _program()
    res = run_bass_kernel_spmd(
        nc, in_maps, core_ids=list(range(N_CORES)), trace=_trace,
        trace_cores=list(range(N_CORES)) if _trace else None,
    )

    out = np.empty((TOK, NOUT), dtype=np.float32)
    for c in range(N_CORES):
        outpk = res.results[c]["outpk"]  # [P, NT*TPC]
        # out[s, t*P + p] = outpk[p, t*TPC + s]
        out[c * TPC : (c + 1) * TPC, :] = (
            outpk.reshape(P, NT, TPC).transpose(2, 1, 0).reshape(TPC, NOUT)
        )
    out = out.reshape(B, S, NOUT)

    if _trace:
        kernel.last_exec_time_ns = res.exec_time_ns
        kernel.last_results = res
    return out


# revision 20
# speedup vs baseline: 1.3126x; 1.0026x over previous
"""Banded-matrix matmul kernel for Trainium2, SPMD over 8 NeuronCores.

Problem: out[b,s,o] = sum_i x[b,s,i] * W[o,i] + bias[o] with W a 4096x4096
band matrix (bandwidth 512 -> W[o,i] != 0 iff |o-i| <= 512), given in COO
form (W_values, rows, cols) with deterministic band ordering.

Strategy:
  - Host: densify W; shard tokens 8-way (data parallel; band + bias
    replicated). All device-side tensors are host-packed partition-major so
    every DMA is a 2D pattern with 8-18KB contiguous per-partition rows
    (SDMA packet overhead amortized; the band's zero padding never moves).
  - Device (per core): out.T[o,s] = W @ x.T per 128-row output tile,
    accumulating over the band's k-tiles in PSUM via float32r TensorEngine
    matmuls (full fp32 data, 1 cycle/row at N=512), bias added during the
    PSUM->SBUF drain on DVE/ACT. x streams through a sliding window of
    4-k-tile groups; W streams in 4-output-tile groups; outputs leave in
    2-output-tile stores.
  - Host: unpack per-core [128, 32*1024] outputs back to [B, S, 4096].
"""

import sys

if "/opt/trn_rl_repo" not in sys.path:
    sys.path.insert(0, "/opt/trn_rl_repo")

import numpy as np

import concourse.bass as bass
import concourse.mybir as mybir
from concourse import tile
from concourse import bass_utils
from concourse.vector_clock import ScopedClock
from concourse.bass_utils import run_bass_kernel_spmd

# ---------------------------------------------------------------- constants
N_CORES = 8
NIN = 4096
NOUT = 4096
BW = 512
B, S = 4, 2048
TOK = B * S            # 8192 tokens
TPC = TOK // N_CORES   # 1024 tokens per core
P = 128                # partitions
NT = NOUT // P         # 32 output tiles of 128 rows
HALF = 512             # moving-operand free size per matmul (4-byte max)

XG = 4                 # k-tiles per x-group       (16KB/partition rows)
WG = 4                 # o-tiles per weight group  (<=18.4KB/partition rows)
OG = 2                 # o-tiles per output store  (8KB/partition rows)
NXG = NT // XG
NWG = NT // WG

# per output tile t: band spans k-tiles [KS[t], KE[t])
KS = [max(0, t - BW // P) for t in range(NT)]
KE = [min(NT, t + BW // P + 1) for t in range(NT)]
NK = [KE[t] - KS[t] for t in range(NT)]
# weight-group layout: group g holds o-tiles [g*WG, (g+1)*WG), each slab
# [P, nk*P] partition-major, concatenated along the free axis
WGNK = [sum(NK[g * WG + i] for i in range(WG)) for g in range(NWG)]
WGOFF = [0] * NWG
for g in range(1, NWG):
    WGOFF[g] = WGOFF[g - 1] + WGNK[g - 1]
WGNK_MAX = max(WGNK)
NK_TOTAL = sum(NK)

COMPUTE_DT = mybir.dt.float16   # halves x/W HBM traffic; ~5e-4 rounding,
                                # fp32 PSUM accumulation; 1 PE cycle/row
COMPUTE_NP = np.float16
OUT_DT = mybir.dt.float32

# ------------------------------------------------- walrus 1-wait workaround
_MAXW = 1


def _split_drain_and_barrier(self, tick_clock, wait_clock):
    nc = self.nc
    probe = nc.sync.nop(nofuse=True, hint="pre_drain_waits")
    wait_clock.add_sem_waits(probe.ins, ScopedClock({None: tick_clock.global_clock}))
    si = probe.ins.sync_info
    waits = list(si.on_wait) if si is not None and si.on_wait else []
    if len(waits) > _MAXW:
        probe.ins.sync_info = mybir.SyncInfo(
            on_wait=waits[:_MAXW],
            on_update=list(si.on_update) if si.on_update else [],
        )
        for i in range(_MAXW, len(waits), _MAXW):
            extra = nc.sync.nop(nofuse=True, hint=f"pre_drain_waits_{i}")
            extra.ins.sync_info = mybir.SyncInfo(
                on_wait=waits[i : i + _MAXW], on_update=[]
            )
    drain_inst = nc.sync.drain()
    wait_clock.add_sem_waits(
        drain_inst.ins, ScopedClock({None: tick_clock.global_clock})
    )
    dsi = drain_inst.ins.sync_info
    dwaits = list(dsi.on_wait) if dsi is not None and dsi.on_wait else []
    if len(dwaits) > _MAXW:
        # the NOPs above ran earlier on the same sequencer and carried them all
        drain_inst.ins.sync_info = mybir.SyncInfo(
            on_wait=[], on_update=list(dsi.on_update) if dsi.on_update else []
        )
    nc.all_engine_barrier()
    popped = nc._tile_sem_poison_stack.pop()
    assert popped is self._sem_poison
    nc.clear_and_free_semaphores(list(self.sems.allocated().values()))
    nc.all_engine_barrier()


tile.TileContext._drain_and_barrier = _split_drain_and_barrier


def fix_multi_waits(nc: bass.Bass) -> None:
    """This walrus build allows only ONE sync wait per instruction. Carry
    extra waits on single-wait NOPs inserted just before, on the same
    engine/sequencer."""
    for bb in nc.m.functions[0].blocks:
        changed = False
        new_insts = []
        for inst in bb.instructions:
            si = inst.sync_info
            waits = list(si.on_wait) if si is not None and si.on_wait else []
            if len(waits) > 1:
                for w in waits[:-1]:
                    nop = mybir.InstNoOp(
                        name=nc.get_next_instruction_name(),
                        engine=inst.engine,
                        bass_nofuse=True,
                        sync_info=mybir.SyncInfo(on_wait=[w], on_update=[]),
                    )
                    new_insts.append(nop)
                inst.sync_info = mybir.SyncInfo(
                    on_wait=[waits[-1]],
                    on_update=list(si.on_update) if si.on_update else [],
                )
                changed = True
            new_insts.append(inst)
        if changed:
            bb.instructions = new_insts


# upload_artifacts reaches an internal blob store not present here; the trace
# path only needs the local files.
bass_utils.upload_artifacts = lambda tmpdir: "local://" + tmpdir


# ---------------------------------------------------------------- device IR
def build_program() -> bass.Bass:
    nc = bass.Bass()
    # all host-packed partition-major (see kernel())
    xpk = nc.declare_dram_parameter("xpk", [P, NT * TPC], COMPUTE_DT, isOutput=False)
    wpk = nc.declare_dram_parameter("wpk", [P, NK_TOTAL * P], COMPUTE_DT, isOutput=False)
    bias = nc.declare_dram_parameter("bias_pk", [P, NT], OUT_DT, isOutput=False)
    outp = nc.declare_dram_parameter("outpk", [P, NT * TPC], OUT_DT, isOutput=True)

    with tile.TileContext(nc) as tc:
        with (
            # fp16 x and W fit SBUF-resident; one buffer per group, no reuse
            tc.tile_pool(name="xp", bufs=1) as xp,
            tc.tile_pool(name="wp", bufs=1) as wp,
            tc.tile_pool(name="op", bufs=3) as op,
            tc.tile_pool(name="bp", bufs=1) as bp,
            tc.tile_pool(name="pp", bufs=4, space="PSUM") as pp,
        ):
            bias_sb = bp.tile([P, NT], OUT_DT)
            nc.sync.dma_start(out=bias_sb[:, :], in_=bias[:, :])

            x_tiles: list = [None] * NXG
            w_tiles: list = [None] * NWG

            def load_xg(g, parts=1):
                # parts>1 fine-grains the load so early k-tile consumers can
                # start before the whole group lands (subtile deps)
                xt = xp.tile([P, XG * TPC], COMPUTE_DT, tag=f"xg{g}", name=f"xg{g}")
                base = g * XG * TPC
                step = XG * TPC // parts
                for i in range(parts):
                    nc.sync.dma_start(
                        out=xt[:, i * step : (i + 1) * step],
                        in_=xpk[:, base + i * step : base + (i + 1) * step],
                    )
                x_tiles[g] = xt

            def load_wg(g, parts=1):
                wt = wp.tile(
                    [P, WGNK[g] * P], COMPUTE_DT, tag=f"wg{g}", name=f"wg{g}",
                )
                bounds = [WGNK[g] * i // parts for i in range(parts + 1)]
                base = WGOFF[g] * P
                for i in range(parts):
                    lo, hi = bounds[i] * P, bounds[i + 1] * P
                    nc.sync.dma_start(
                        out=wt[:, lo:hi], in_=wpk[:, base + lo : base + hi]
                    )
                w_tiles[g] = wt

            # the critical first bytes, fine-grained: o-tile 0 only needs
            # slab t0 (first 5 k-units of wg0) + x k-tiles 0..4
            load_wg(0, parts=5)
            load_xg(0, parts=4)
            load_xg(1, parts=4)

            ot = None
            for t in range(NT):
                gw = t // WG
                if w_tiles[gw] is None:
                    load_wg(gw)
                for g in range(KS[t] // XG, (KE[t] - 1) // XG + 1):
                    if x_tiles[g] is None:
                        load_xg(g)

                # slab offset of o-tile t inside its weight group
                off = sum(NK[gw * WG + i] for i in range(t - gw * WG))
                wt = w_tiles[gw]

                ps0 = pp.tile([P, HALF], mybir.dt.float32, name=f"ps0_{t}", tag="ps0")
                ps1 = pp.tile([P, HALF], mybir.dt.float32, name=f"ps1_{t}", tag="ps1")
                for j in range(NK[t]):
                    k = KS[t] + j
                    lhsT = wt[:, (off + j) * P : (off + j + 1) * P]
                    xg = x_tiles[k // XG]
                    xbase = (k % XG) * TPC
                    nc.tensor.matmul(
                        ps0[:, :], lhsT, xg[:, xbase : xbase + HALF],
                        start=(j == 0), stop=(j == NK[t] - 1),
                    )
                    nc.tensor.matmul(
                        ps1[:, :], lhsT, xg[:, xbase + HALF : xbase + TPC],
                        start=(j == 0), stop=(j == NK[t] - 1),
                    )

                if t % OG == 0:
                    ot = op.tile([P, OG * TPC], OUT_DT, name=f"ot{t}", tag="ot")
                obase = (t % OG) * TPC
                bias_col = bias_sb[:, t : t + 1]
                nc.vector.tensor_scalar_add(
                    ot[:, obase : obase + HALF], ps0[:, :], bias_col
                )
                nc.scalar.activation(
                    ot[:, obase + HALF : obase + TPC], ps1[:, :],
                    mybir.ActivationFunctionType.Identity, bias=bias_col,
                )
                if t % OG == OG - 1:
                    nc.sync.dma_start(
                        out=outp[:, (t - OG + 1) * TPC : (t + 1) * TPC],
                        in_=ot[:, :],
                    )

    fix_multi_waits(nc)
    return nc


_PROGRAM_CACHE: bass.Bass | None = None


def _program() -> bass.Bass:
    global _PROGRAM_CACHE
    if _PROGRAM_CACHE is None:
        _PROGRAM_CACHE = build_program()
    return _PROGRAM_CACHE


# --------------------------------------------------------------- host side
def _pack_weights(W_values, rows, cols) -> np.ndarray:
    W = np.zeros((NOUT, NIN), dtype=np.float32)
    W[rows, cols] = W_values
    slabs = []
    for t in range(NT):
        # slab[p, j*P + o] = W[t*P + o, (KS[t]+j)*P + p]
        blk = W[t * P : (t + 1) * P, KS[t] * P : KE[t] * P]  # [o, nk*P]
        slab = blk.reshape(P, NK[t], P).transpose(2, 1, 0).reshape(P, NK[t] * P)
        slabs.append(slab)
    return np.ascontiguousarray(
        np.concatenate(slabs, axis=1), dtype=COMPUTE_NP
    )  # [P, NK_TOTAL*P]


def kernel(x, W_values, bias, rows, cols, _trace=False):
    x = np.asarray(x, dtype=np.float32)
    W_values = np.asarray(W_values, dtype=np.float32)
    bias = np.asarray(bias, dtype=np.float32)
    rows = np.asarray(rows)
    cols = np.asarray(cols)

    x2d = x.reshape(TOK, NIN)
    wpk = _pack_weights(W_values, rows, cols)
    bias_pk = np.ascontiguousarray(bias.reshape(NT, P).T)

    in_maps = []
    for c in range(N_CORES):
        xs = x2d[c * TPC : (c + 1) * TPC, :]  # [TPC, NIN]
        # xpk[p, j*TPC + s] = xs[s, j*P + p]
        xpk = np.ascontiguousarray(
            xs.reshape(TPC, NT, P).transpose(2, 1, 0).reshape(P, NT * TPC),
            dtype=COMPUTE_NP,
        )
        in_maps.append({"xpk": xpk, "wpk": wpk, "bias_pk": bias_pk})

    nc = _program()
    res = run_bass_kernel_spmd(
        nc, in_maps, core_ids=list(range(N_CORES)), trace=_trace,
        trace_cores=list(range(N_CORES)) if _trace else None,
    )

    out = np.empty((TOK, NOUT), dtype=np.float32)
    for c in range(N_CORES):
        outpk = res.results[c]["outpk"]  # [P, NT*TPC]
        # out[s, t*P + p] = outpk[p, t*TPC + s]
        out[c * TPC : (c + 1) * TPC, :] = (
            outpk.reshape(P, NT, TPC).transpose(2, 1, 0).reshape(TPC, NOUT)
        )
    out = out.reshape(B, S, NOUT)

    if _trace:
        kernel.last_exec_time_ns = res.exec_time_ns
        kernel.last_results = res
    return out
